# revision 9
# baseline (speedup 1.0000x reference)
"""GCN layer on 8 TRN2 NeuronCores.

out = D_in^{-1/2} * A^T * D_out^{-1/2} * X  for a random graph with
N=100000 nodes, d=32 features, E=1600000 edges.

Strategy (dst-sharded, no collectives):
 - Host sorts edges by dst and gives core c the edges whose dst lies in
   [12500c, 12500(c+1)).  Each core owns its output slice outright, so no
   all-reduce is needed.  The host also lays out the raw node_f bytes per
   edge slot (transport layout only -- all arithmetic, including both
   degree rsqrt normalizations, happens on device; the host ships integer
   degree counts and integer CSR boundaries).
 - Edges are packed into 128-edge tiles grouped in 32-dst-node windows
   (5 tiles per window, fixed).  On device: messages are scaled by
   rsqrt(out_deg) (DVE/ACT), a DVE is_equal against an iota builds the
   per-tile selection matrix, and PE matmuls (sel^T @ msgs) accumulate
   each window's segment sum in PSUM.  A final DVE multiply applies
   rsqrt(in_deg) computed from dst CSR boundaries.

Note: per-edge *device-side* table gather was implemented and abandoned:
multi-index indirect DMA emits wrong descriptors on this runtime and the
InstDMAGatherAnt/InstDMAScatterAddAnt ucode paths crash the exec unit
(NRT_EXEC_UNIT_UNRECOVERABLE), so the gather moved to host layout.
"""

import numpy as np

P = 128
D = 32
NN = 100000
N_CORES = 8
SLICE = 12500                   # dst nodes per core
W = 32                          # dst window (PSUM rows per window)
TPW = 5                         # tiles per window (5*128 = 640 edge slots)
WINDOWS = 392                   # 392*32 = 12544 slots >= 12500
TT = WINDOWS * TPW              # 1960 edge tiles per core
KTILES = 98                     # 12544/128 output tiles
SELU = 7                        # tiles per sel-build op (1960 = 280*7)

_COMPILED = None


def _build_program():
    from contextlib import ExitStack
    from concourse import bass, bacc, mybir
    from concourse import tile

    f32 = mybir.dt.float32
    bf16 = mybir.dt.bfloat16
    i32 = mybir.dt.int32
    Alu = mybir.AluOpType
    Act = mybir.ActivationFunctionType

    nc = bacc.Bacc()

    msgs = nc.declare_dram_parameter("msgs", [P, TT * D], bf16, isOutput=False)
    edeg = nc.declare_dram_parameter("edeg", [P, TT], i32, isOutput=False)
    db_lo = nc.declare_dram_parameter("db_lo", [P, KTILES], i32, isOutput=False)
    db_hi = nc.declare_dram_parameter("db_hi", [P, KTILES], i32, isOutput=False)
    dr = nc.declare_dram_parameter("dr", [P, TT], bf16, isOutput=False)
    iota = nc.declare_dram_parameter("iota", [P, W * SELU], bf16, isOutput=False)
    outp = nc.declare_dram_parameter("out", [P, KTILES * D], f32, isOutput=True)

    with ExitStack() as ctx:
        tc = ctx.enter_context(tile.TileContext(nc))
        constp = ctx.enter_context(tc.tile_pool(name="const", bufs=1))
        selp = ctx.enter_context(tc.tile_pool(name="sel", bufs=4))
        psp = ctx.enter_context(tc.tile_pool(name="ps", bufs=4, space="PSUM"))
        aggp = ctx.enter_context(tc.tile_pool(name="agg", bufs=1))

        # ---- out-degree rsqrt per edge slot: rsqd = rsqrt(max(deg,1)) ----
        edeg_t = constp.tile([P, TT], i32)
        nc.sync.dma_start(edeg_t[:], edeg[:, :])
        degf_t = constp.tile([P, TT], f32)
        nc.vector.tensor_copy(degf_t[:], edeg_t[:])
        nc.vector.tensor_scalar_max(degf_t[:], degf_t[:], 1.0)
        dinv_t = constp.tile([P, TT], f32)
        nc.vector.reciprocal(dinv_t[:], degf_t[:])
        rsqd_t = constp.tile([P, TT], bf16)
        nc.scalar.activation(rsqd_t[:], dinv_t[:], Act.Sqrt)

        # ---- in-degree rsqrt per output slot ----
        dbl_t = constp.tile([P, KTILES], i32)
        nc.sync.dma_start(dbl_t[:], db_lo[:, :])
        dbh_t = constp.tile([P, KTILES], i32)
        nc.sync.dma_start(dbh_t[:], db_hi[:, :])
        icnt_i = constp.tile([P, KTILES], i32)
        nc.vector.tensor_tensor(icnt_i[:], dbh_t[:], dbl_t[:], Alu.subtract)
        icnt_f = constp.tile([P, KTILES], f32)
        nc.vector.tensor_copy(icnt_f[:], icnt_i[:])
        nc.vector.tensor_scalar_max(icnt_f[:], icnt_f[:], 1.0)
        iinv_t = constp.tile([P, KTILES], f32)
        nc.vector.reciprocal(iinv_t[:], icnt_f[:])
        rsqin_t = constp.tile([P, KTILES], f32)
        nc.scalar.activation(rsqin_t[:], iinv_t[:], Act.Sqrt)

        dr_t = constp.tile([P, TT], bf16)
        nc.sync.dma_start(dr_t[:], dr[:, :])
        iota_t = constp.tile([P, W * SELU], bf16)
        nc.sync.dma_start(iota_t[:], iota[:, :])

        # ---- load raw messages, scale by rsqrt(out_deg) in chunks ----
        msg_t = constp.tile([P, TT * D], bf16)
        NCH = 8
        CH = TT // NCH  # 245 tiles per chunk
        for c in range(NCH):
            sl = slice(c * CH * D, (c + 1) * CH * D)
            nc.sync.dma_start(msg_t[:, sl], msgs[:, sl])
            nc.vector.tensor_tensor(
                msg_t[:, sl].rearrange("p (t f) -> p t f", f=D),
                msg_t[:, sl].rearrange("p (t f) -> p t f", f=D),
                rsqd_t[:, c * CH : (c + 1) * CH].unsqueeze(2).to_broadcast([P, CH, D]),
                Alu.mult,
            )

        agg_t = aggp.tile([P, KTILES * D], f32)

        # ---- selection matrices + windowed PSUM matmuls ----
        ps_t = None
        for bsel in range(TT // SELU):
            t0 = bsel * SELU
            sel_t = selp.tile([P, W * SELU], bf16)
            nc.vector.tensor_tensor(
                sel_t[:].rearrange("p (j u) -> p j u", u=SELU),
                dr_t[:, t0 : t0 + SELU].unsqueeze(1).to_broadcast([P, W, SELU]),
                iota_t[:].rearrange("p (j u) -> p j u", u=SELU),
                Alu.is_equal,
            )
            sel_v = sel_t[:].rearrange("p (j u) -> p u j", u=SELU)
            for u in range(SELU):
                t = t0 + u
                w = t // TPW
                k = w // 4
                r0 = W * (w % 4)
                if t % (4 * TPW) == 0:
                    ps_t = psp.tile([P, D], f32)
                nc.tensor.matmul(
                    ps_t[r0 : r0 + W, :],
                    sel_v[:, u, :],
                    msg_t[:, t * D : (t + 1) * D],
                    start=(t % TPW == 0),
                    stop=(t % TPW == TPW - 1),
                    skip_group_check=True,
                    tile_position=(0, r0),
                )
                if t % (4 * TPW) == 4 * TPW - 1:
                    nc.vector.tensor_copy(agg_t[:, k * D : (k + 1) * D], ps_t[:, :])

        # ---- final scale + store ----
        out_t = aggp.tile([P, KTILES * D], f32, tag="outt")
        nc.vector.tensor_tensor(
            out_t[:].rearrange("p (k f) -> p k f", f=D),
            agg_t[:].rearrange("p (k f) -> p k f", f=D),
            rsqin_t[:].unsqueeze(2).to_broadcast([P, KTILES, D]),
            Alu.mult,
        )
        nc.sync.dma_start(outp[:, :], out_t[:])

    nc.compile()
    return nc


def _host_shard(node_f, src, dst):
    """Index bookkeeping + transport layout: sort/bucket edges, lay out raw
    node_f bytes per edge slot, ship integer degree counts/boundaries."""
    from ml_dtypes import bfloat16

    node_f = np.asarray(node_f, dtype=np.float32)
    src = np.asarray(src).astype(np.int64)
    dst = np.asarray(dst).astype(np.int64)

    nf16 = node_f.astype(bfloat16)
    out_deg = np.bincount(src, minlength=NN).astype(np.int32)  # integer counts

    order = np.argsort(dst, kind="stable")
    dsts = dst[order]
    srcs = src[order]
    cb = np.searchsorted(dsts, np.arange(0, NN + 1, SLICE))

    iota_arr = np.tile(
        np.repeat(np.arange(W, dtype=np.float32), SELU), (P, 1)
    ).astype(bfloat16)

    in_maps = []
    for c in range(N_CORES):
        lo, hi = cb[c], cb[c + 1]
        slot = (dsts[lo:hi] - c * SLICE).astype(np.int64)
        esrc = srcs[lo:hi]
        n = slot.shape[0]

        wv = slot // W
        cnt_w = np.bincount(wv, minlength=WINDOWS)
        if cnt_w.max() > TPW * P:
            raise ValueError(
                f"window overflow: {cnt_w.max()} edges > {TPW * P} capacity"
            )
        ws = np.zeros(WINDOWS, dtype=np.int64)
        np.cumsum(cnt_w[:-1], out=ws[1:])
        q = np.arange(n, dtype=np.int64) - ws[wv]
        tt = TPW * wv + q // P
        pp = q % P

        msgs_arr = np.zeros((P, TT, D), dtype=bfloat16)
        msgs_arr[pp, tt] = nf16[esrc]
        edeg_arr = np.ones((P, TT), dtype=np.int32)
        edeg_arr[pp, tt] = out_deg[esrc]
        dr_arr = np.full((P, TT), float(W), dtype=np.float32)
        dr_arr[pp, tt] = (slot - W * wv).astype(np.float32)

        dbl = np.searchsorted(slot, np.arange(KTILES * P)).astype(np.int32)
        dbh = np.searchsorted(slot, np.arange(1, KTILES * P + 1)).astype(np.int32)
        db_lo = np.ascontiguousarray(dbl.reshape(KTILES, P).T)
        db_hi = np.ascontiguousarray(dbh.reshape(KTILES, P).T)

        in_maps.append(
            {
                "msgs": msgs_arr.reshape(P, TT * D),
                "edeg": edeg_arr,
                "db_lo": db_lo,
                "db_hi": db_hi,
                "dr": dr_arr.astype(bfloat16),
                "iota": iota_arr,
            }
        )
    return in_maps


def kernel(node_f, src, dst):
    from concourse.bass_utils import run_bass_kernel_spmd

    global _COMPILED
    if _COMPILED is None:
        _COMPILED = _build_program()
    nc = _COMPILED

    in_maps = _host_shard(node_f, src, dst)
    res = run_bass_kernel_spmd(nc, in_maps, list(range(N_CORES)))
    kernel.last_results = res

    out = np.empty((NN, D), dtype=np.float32)
    for c in range(N_CORES):
        o = np.asarray(res.results[c]["out"], dtype=np.float32)
        o = o.reshape(P, KTILES, D).transpose(1, 0, 2).reshape(KTILES * P, D)
        out[c * SLICE : (c + 1) * SLICE] = o[:SLICE]
    return out
